# revision 2
# baseline (speedup 1.0000x reference)
"""Trainium2 Bass kernel for BeliefPlausibility (Dempster-Shafer bel/pl maps).

Problem: input [4, 384, 1248, 7] fp32 (6 singleton masses + omega per pixel).
Output: tuple (bel, pl), each [4, 384, 1248, 64] fp32 where, per pixel with
masses m_0..m_5 and omega w:
    bel[q] = sum_c m_c * ((q >> c) & 1)  for q in 1..62;  bel[0]=0, bel[63]=1
    pl[q]  = bel[q] + w                  for q in 1..62;  pl[0]=0,  pl[63]=1

Strategy (pure data parallel over 8 cores, no cross-core communication):
  - The kernel is HBM-bound: the two outputs are 64x the input.  All device
    I/O is bf16 (inputs quantized on host, outputs upcast on host).  All
    values are positive sums (no cancellation), so worst-case relative error
    is ~3 bf16 round-offs (~6e-3), well inside the 2e-2 gate.  bf16 (not
    fp16) because fp16's subnormal range [6e-8, 6e-5] would blow up the
    relative error of the smallest singleton masses.
  - Flatten pixels; each core gets 239,616 pixels as [117, 128, 112]
    (117 supertiles x 128 partitions x 16 pixels x 7 channels).  Input HBM
    layout is partition-major [128, 117*112] so a 9-supertile input DMA
    reads 2016B-contiguous lines per partition (line-rate HBM bursts).
  - Per supertile: PE-transpose to channels-on-partitions; two bf16 matmuls
    (1 cycle/row) against a constant [112, 1024] membership matrix produce
    PSUM [128, 512] f32 already in the per-pixel bel layout (8 pixel-groups
    x 64 output columns); ACT copies bel PSUM->SBUF bf16; DVE derives
    pl = bel + omega with a zero-stride broadcast AP into the same SBUF
    buffer; one contiguous 512 KB DMA stores bel|pl together.
  - Output SBUF buffers are persistent slices of one tensor; the constant
    columns (bel[63]=1, pl[0]=0, pl[63]=1) are memset ONCE before the loop
    and never rewritten (per-tile ops only touch cols 0..62 / 1..62).
  - Walrus allows only ONE sync-wait on a self-weight-loading Matmult, so a
    tiny "absorber" matmul (into a dummy PSUM tile) observes the in-DMA
    tick first, keeping every real Matmult at <=1 wait.  PSUM is read by a
    single engine per tensor (ACT for the matmul banks and the transpose
    bank) for the same reason.
"""

import sys

import numpy as np
from ml_dtypes import bfloat16

if "concourse" not in sys.modules:
    try:
        import concourse  # noqa: F401
    except ImportError:
        sys.path.insert(0, "/opt/trn_rl_repo")

import concourse.bacc as bacc
import concourse.bass as bass
import concourse.mybir as mybir
import concourse.tile as tile
from concourse.bass_utils import run_bass_kernel_spmd

F32 = mybir.dt.float32
BF16 = mybir.dt.bfloat16

N_CORES = 8
PX_TOTAL = 4 * 384 * 1248          # 1,916,928 pixels
PX_CORE = PX_TOTAL // N_CORES      # 239,616
PX_PART = 16                       # pixels per partition per supertile
PX_TILE = 128 * PX_PART            # 2048 pixels per supertile
N_TILES = PX_CORE // PX_TILE       # 117
N_CH = 7                           # 6 singletons + omega
N_SUB = 64                         # output positions per pixel
K_ROWS = PX_PART * N_CH            # 112 channel rows
GROUPS_PER_MM = 8                  # pixel-groups covered by one matmul
N_MM = PX_PART // GROUPS_PER_MM    # 2 matmuls per supertile
OUT_COLS = 2 * PX_PART * N_SUB     # 2048: bel block | pl block per tile
IN_GRP = 9                         # supertiles per input DMA (117 = 9*13)
OUT_BUFS = 6                       # persistent output SBUF buffers


def _weight_matrix() -> np.ndarray:
    """[112, 1024]: W[7j+c, 512h+64g+q] = (q>>c)&1 for j=8h+g, q in 1..62,
    c in 0..5.  Columns (g,0) and (g,63) stay zero (written separately)."""
    w = np.zeros((K_ROWS, N_MM * 512), np.float32)
    for h in range(N_MM):
        for g in range(GROUPS_PER_MM):
            j = GROUPS_PER_MM * h + g
            col0 = 512 * h + 64 * g
            for q in range(1, 63):
                for c in range(6):
                    if (q >> c) & 1:
                        w[7 * j + c, col0 + q] = 1.0
    return w


def build_program(n_tiles: int = N_TILES, reps: int = 1) -> bass.Bass:
    # Bacc (not plain Bass): its compile() runs generate_event_semaphores,
    # which splits multi-semaphore waits into standalone event-sem
    # instructions (TRN2 allows at most one wait per instruction).
    nc = bacc.Bacc("TRN2")

    x = nc.dram_tensor("x", (128, n_tiles * K_ROWS), BF16,
                       kind="ExternalInput")
    out = nc.dram_tensor("out", (n_tiles, 128, OUT_COLS), BF16,
                         kind="ExternalOutput")

    w_dram = nc.inline_tensor(_weight_matrix(), name="wmat")
    id_dram = nc.inline_tensor(np.eye(128, dtype=np.float32), name="ident")

    with tile.TileContext(nc) as tc:
        with (
            tc.tile_pool(name="const", bufs=1) as cpool,
            tc.tile_pool(name="inp", bufs=3) as inpool,
            tc.tile_pool(name="tp", bufs=4) as tpool,
            tc.tile_pool(name="om", bufs=4) as ompool,
            tc.tile_pool(name="psT", bufs=3, space="PSUM") as psTpool,
            tc.tile_pool(name="psM", bufs=1, space="PSUM") as psMpool,
            tc.tile_pool(name="psD", bufs=1, space="PSUM") as psDpool,
        ):
            # Stage the constants through an ACT copy (f32 -> bf16; all
            # values are exact 0/1): matmuls reading an ACT-produced tensor
            # can merge that dep with their other ACT deps into a single
            # semaphore wait (walrus allows only one sync-wait on Matmults).
            wstage = cpool.tile([K_ROWS, N_MM * 512], F32)
            nc.sync.dma_start(wstage[:], w_dram[:])
            wmat = cpool.tile([K_ROWS, N_MM * 512], BF16)
            nc.scalar.copy(wmat[:], wstage[:])
            istage = cpool.tile([128, 128], F32)
            nc.sync.dma_start(istage[:], id_dram[:])
            ident = cpool.tile([128, 128], BF16)
            nc.scalar.copy(ident[:], istage[:])
            dum = psDpool.tile([1, 1], F32)
            # One persistent 4-bank PSUM tensor, slices cycled manually:
            # avoids pool-release machinery so matmul slot-reuse deps stay
            # byte-range (same-engine WAW = program order, reader WAR = ACT).
            ps_all = psMpool.tile([128, 4 * 512], F32)

            # Persistent output buffers (manually cycled slices).  Constant
            # columns are initialized once, outside the loop; the per-tile
            # copies/adds never touch them.
            outbuf = cpool.tile([128, OUT_BUFS * OUT_COLS], BF16)
            for b in range(OUT_BUFS):
                ob = outbuf[:, b * OUT_COLS:(b + 1) * OUT_COLS]
                bel3 = ob[:, 0:PX_PART * N_SUB].rearrange(
                    "p (g q) -> p g q", q=N_SUB)
                pl3 = ob[:, PX_PART * N_SUB:OUT_COLS].rearrange(
                    "p (g q) -> p g q", q=N_SUB)
                nc.vector.memset(bel3[:, :, 63:64], 1.0)
                nc.vector.memset(pl3[:, :, 0:1], 0.0)
                nc.vector.memset(pl3[:, :, 63:64], 1.0)

            in_t = None
            for g in range(reps * n_tiles):
                t = g % n_tiles
                j = t % IN_GRP
                if j == 0:
                    in_t = inpool.tile([128, IN_GRP * K_ROWS], BF16)
                    nc.sync.dma_start(
                        in_t[:], x[:, t * K_ROWS:(t + IN_GRP) * K_ROWS])
                    # Absorb the in-DMA wait on PE so the transposes (max
                    # one sync-wait per Matmult) stay at <=1 wait.
                    nc.tensor.matmul(dum[:], in_t[0:1, 0:1], in_t[0:1, 0:1])
                base = j * K_ROWS

                ps_t = psTpool.tile([K_ROWS, 128], BF16)
                nc.tensor.transpose(ps_t[:], in_t[:, base:base + K_ROWS],
                                    ident[:])

                # `that` is produced on ACT so the matmuls' deps (data RAW +
                # weight RAW + PSUM-slot release, whose reader is also ACT)
                # merge into a single ACT semaphore wait.
                that = tpool.tile([K_ROWS, 128], BF16)
                nc.scalar.copy(that[:], ps_t[:])

                # Stage the omega channels through DVE: the pl tensor_add
                # then reads only DVE- and ACT-produced operands.
                omg = ompool.tile([128, PX_PART], BF16)
                nc.vector.tensor_copy(
                    omg[:], in_t[:, base + 6:base + K_ROWS:7])

                b = g % OUT_BUFS
                ob = outbuf[:, b * OUT_COLS:(b + 1) * OUT_COLS]
                bel3 = ob[:, 0:PX_PART * N_SUB].rearrange(
                    "p (g q) -> p g q", q=N_SUB)
                pl3 = ob[:, PX_PART * N_SUB:OUT_COLS].rearrange(
                    "p (g q) -> p g q", q=N_SUB)

                for h in range(N_MM):
                    slot = (2 * g + h) % 4
                    ps = ps_all[:, 512 * slot:512 * (slot + 1)]
                    nc.tensor.matmul(ps, that[:],
                                     wmat[:, 512 * h:512 * (h + 1)])
                    ps3 = ps.rearrange("p (g q) -> p g q", q=N_SUB)
                    gsl = slice(GROUPS_PER_MM * h, GROUPS_PER_MM * (h + 1))

                    # bel columns 0..62 of each group: ACT copy PSUM->SBUF
                    # (f32 -> bf16; col 0 comes from the all-zero W column).
                    nc.scalar.copy(bel3[:, gsl, 0:63], ps3[:, :, 0:63])

                    # pl cols 1..62: bel + omega (zero-stride broadcast)
                    om = omg[:, GROUPS_PER_MM * h:GROUPS_PER_MM * (h + 1)]
                    om = bass.AP(om.tensor, om.offset, om.ap + [[0, 62]])
                    nc.vector.tensor_add(pl3[:, gsl, 1:63],
                                         bel3[:, gsl, 1:63], om)

                nc.sync.dma_start(out[t], ob)

    nc.compile()
    return nc


_NC_CACHE: dict[int, bass.Bass] = {}


def _get_program(n_tiles: int) -> bass.Bass:
    if n_tiles not in _NC_CACHE:
        _NC_CACHE[n_tiles] = build_program(n_tiles)
    return _NC_CACHE[n_tiles]


def run_on_cores(x_flat: np.ndarray, **run_kwargs):
    """x_flat: [PX_TOTAL, 7] bf16 (or castable). Returns (bel, pl) each
    [PX_TOTAL, 64] fp32, plus the raw BassKernelResults as third element."""
    if x_flat.dtype != bfloat16:
        x_flat = x_flat.astype(bfloat16)
    nc = _get_program(N_TILES)
    in_maps = []
    for c in range(N_CORES):
        shard = x_flat[c * PX_CORE:(c + 1) * PX_CORE].reshape(
            N_TILES, 128, K_ROWS)
        # partition-major HBM layout: [128, N_TILES * K_ROWS]
        shard = np.ascontiguousarray(shard.transpose(1, 0, 2)).reshape(
            128, N_TILES * K_ROWS)
        in_maps.append({"x": shard})
    rr = run_bass_kernel_spmd(nc, in_maps, core_ids=list(range(N_CORES)),
                              **run_kwargs)
    bel = np.empty((PX_TOTAL, N_SUB), np.float32)
    pl = np.empty((PX_TOTAL, N_SUB), np.float32)
    half = PX_PART * N_SUB
    for c, res in enumerate(rr.results):
        sl = slice(c * PX_CORE, (c + 1) * PX_CORE)
        o = np.asarray(res["out"])          # [N_TILES, 128, 2048] bf16
        bel[sl] = o[:, :, :half].astype(np.float32).reshape(PX_CORE, N_SUB)
        pl[sl] = o[:, :, half:].astype(np.float32).reshape(PX_CORE, N_SUB)
    return bel, pl, rr


def kernel(inputs: np.ndarray):
    inputs = np.ascontiguousarray(np.asarray(inputs, dtype=np.float32))
    b, hh, ww, ch = inputs.shape
    x_flat = inputs.reshape(-1, ch).astype(bfloat16)
    bel, pl, _ = run_on_cores(x_flat)
    return (bel.reshape(b, hh, ww, N_SUB), pl.reshape(b, hh, ww, N_SUB))


# revision 26
# speedup vs baseline: 1.0382x; 1.0382x over previous
"""Trainium2 Bass kernel for BeliefPlausibility (Dempster-Shafer bel/pl maps).

Problem: input [4, 384, 1248, 7] fp32 (6 singleton masses + omega per pixel).
Output: tuple (bel, pl), each [4, 384, 1248, 64] fp32 where, per pixel with
masses m_0..m_5 and omega w:
    bel[q] = sum_c m_c * ((q >> c) & 1)  for q in 1..62;  bel[0]=0, bel[63]=1
    pl[q]  = bel[q] + w                  for q in 1..62;  pl[0]=0,  pl[63]=1

Strategy (pure data parallel over 8 cores, no cross-core communication):
  - The kernel is HBM-bound: the two outputs are 64x the input.  All device
    I/O is bf16 (inputs quantized on host, outputs upcast on host).  All
    values are positive sums (no cancellation), so worst-case relative error
    is ~3 bf16 round-offs (~1e-2), inside the 2e-2 gate.  bf16 (not fp16)
    because fp16's subnormal range [6e-8, 6e-5] would blow up the relative
    error of the smallest singleton masses.
  - Flatten pixels; each core gets 239,616 pixels as 117 supertiles of
    (128 partition-pixels x 16 group-pixels x 7 channels).  The HOST
    pre-transposes each supertile to channels-on-partitions [112, 128]
    (plus a tiny pixel-major omega side tensor), so the device needs NO
    transpose at all: a supertile slice of the group load is directly the
    stationary matmul operand.
  - Per supertile: two bf16 matmuls against a constant [112, 1024]
    membership matrix produce PSUM [128, 512] f32 already in the per-pixel
    bel layout (8 pixel-groups x 64 output columns); ACT copies bel h0 and
    DVE copies bel h1 PSUM->SBUF bf16 (Pool cannot access PSUM); DVE/Pool
    derive pl = bel + omega with zero-stride broadcast APs; one contiguous
    512 KB DMA stores bel|pl together.
  - Engine budget per tile vs the ~1.55us DMA-store cadence: PE 1.47us
    cold / 0.74us warm, ACT ~0.75us, DVE ~1.35us, Pool ~1.37us — the
    output DMA is the pacer.  pl adds + store are emitted one tile behind
    the bel copies so Bacc's hoisted event-sem waits only ever reference
    the previous iteration's work.
  - Input loads ride the ACT HWDGE ring, output stores the SP ring:
    sharing one ring would head-of-line-block loads behind stores still
    waiting on compute semaphores.
  - The PE HAM clock gate runs the PE at 1.2 GHz until it sees ~3.4us of
    sustained activity; a startup burst plus periodic 4-matmul refresher
    bursts (into a dedicated junk PSUM bank) keep it at 2.4 GHz.
"""

import sys

import numpy as np
from ml_dtypes import bfloat16

if "concourse" not in sys.modules:
    try:
        import concourse  # noqa: F401
    except ImportError:
        sys.path.insert(0, "/opt/trn_rl_repo")

import concourse.bacc as bacc
import concourse.bass as bass
import concourse.mybir as mybir
import concourse.tile as tile
from concourse.bass_utils import run_bass_kernel_spmd

F32 = mybir.dt.float32
BF16 = mybir.dt.bfloat16

N_CORES = 8
PX_TOTAL = 4 * 384 * 1248          # 1,916,928 pixels
PX_CORE = PX_TOTAL // N_CORES      # 239,616
PX_PART = 16                       # pixels per partition per supertile
PX_TILE = 128 * PX_PART            # 2048 pixels per supertile
N_TILES = PX_CORE // PX_TILE       # 117
N_CH = 7                           # 6 singletons + omega
N_SUB = 64                         # output positions per pixel
K_ROWS = PX_PART * N_CH            # 112 channel rows
GROUPS_PER_MM = 8                  # pixel-groups covered by one matmul
N_MM = PX_PART // GROUPS_PER_MM    # 2 matmuls per supertile
OUT_COLS = 2 * PX_PART * N_SUB     # 2048: bel block | pl block per tile
IN_GRP = 9                         # supertiles per input DMA (117 = 9*13)
IN_BUFS = 3                        # input group buffers in flight
OUT_BUFS = 10                      # persistent output SBUF buffers
WARM_EVERY = 24                    # tiles between PE warm refresher bursts


def _weight_matrix() -> np.ndarray:
    """[112, 1024]: W[7j+c, 512h+64g+q] = (q>>c)&1 for j=8h+g, q in 1..62,
    c in 0..5.  Columns (g,0) and (g,63) stay zero (written separately)."""
    w = np.zeros((K_ROWS, N_MM * 512), np.float32)
    for h in range(N_MM):
        for g in range(GROUPS_PER_MM):
            j = GROUPS_PER_MM * h + g
            col0 = 512 * h + 64 * g
            for q in range(1, 63):
                for c in range(6):
                    if (q >> c) & 1:
                        w[7 * j + c, col0 + q] = 1.0
    return w


def build_program(n_tiles: int = N_TILES, reps: int = 1) -> bass.Bass:
    igrp = IN_GRP if n_tiles % IN_GRP == 0 else 1
    # Bacc (not plain Bass): its compile() runs generate_event_semaphores,
    # which splits multi-semaphore waits into standalone event-sem
    # instructions (TRN2 allows at most one wait per instruction).
    nc = bacc.Bacc("TRN2")

    # Host-pretransposed inputs: xt[7j+c, 128t+p] = mass c of pixel
    # (t, p, j); xo[p, 16t+j] = omega of pixel (t, p, j).
    xt = nc.dram_tensor("xt", (K_ROWS, n_tiles * 128), BF16,
                        kind="ExternalInput")
    xo = nc.dram_tensor("xo", (128, n_tiles * PX_PART), BF16,
                        kind="ExternalInput")
    out = nc.dram_tensor("out", (n_tiles, 128, OUT_COLS), BF16,
                         kind="ExternalOutput")

    w_dram = nc.inline_tensor(_weight_matrix(), name="wmat")

    n_total = reps * n_tiles
    n_grp = (n_total + igrp - 1) // igrp
    grp_per_pass = n_tiles // igrp

    with tile.TileContext(nc) as tc:
        with (
            tc.tile_pool(name="const", bufs=1) as cpool,
            tc.tile_pool(name="psM", bufs=1, space="PSUM") as psMpool,
            tc.tile_pool(name="psD", bufs=1, space="PSUM") as psDpool,
        ):
            # Persistent input group buffers, cycled manually.  Group K is
            # issued at the head of group K-2 (3 buffers keep K-2..K
            # alive), so data always lands well before the matmuls read it.
            inbuf = cpool.tile([K_ROWS, IN_BUFS * igrp * 128], BF16)
            ombuf = cpool.tile([128, IN_BUFS * igrp * PX_PART], BF16)
            gdat: dict[int, object] = {}
            gomg: dict[int, object] = {}

            def ensure_group(G: int):
                if G >= n_grp or G in gdat:
                    return
                b = G % IN_BUFS
                ib = inbuf[:, b * igrp * 128:(b + 1) * igrp * 128]
                ob = ombuf[:, b * igrp * PX_PART:(b + 1) * igrp * PX_PART]
                Gp = G % grp_per_pass
                nc.scalar.dma_start(
                    ib, xt[:, Gp * igrp * 128:(Gp + 1) * igrp * 128])
                nc.scalar.dma_start(
                    ob, xo[:, Gp * igrp * PX_PART:(Gp + 1) * igrp * PX_PART])
                gdat[G] = ib
                gomg[G] = ob

            ensure_group(0)
            ensure_group(1)

            # Stage the weights through an ACT copy (f32 -> bf16; all
            # values are exact 0/1).
            wstage = cpool.tile([K_ROWS, N_MM * 512], F32)
            nc.sync.dma_start(wstage[:], w_dram[:])
            wmat = cpool.tile([K_ROWS, N_MM * 512], BF16)
            nc.scalar.copy(wmat[:], wstage[:])

            # Six matmul PSUM slots cycled manually (slot reuse distance =
            # 3 tiles, so the slot-release WAR always references copies
            # finished long ago).  Even slots are read by ACT, odd slots
            # by DVE — one reader engine per slot keeps matmul sync-waits
            # merged.  `dum` is a dedicated junk bank for HAM warm bursts
            # (same-engine WAW only, so the bursts never wait on anything).
            ps_all = psMpool.tile([128, 6 * 512], F32)
            dum = psDpool.tile([128, 512], F32)

            def warm_burst(n: int):
                for _ in range(n):
                    nc.tensor.matmul(dum[:], wmat[:, 0:128], wmat[:, 0:512])

            warm_burst(12)

            # Persistent output buffers (manually cycled slices).  Constant
            # columns are initialized once, outside the loop; the per-tile
            # copies/adds never touch them.
            outbuf = cpool.tile([128, OUT_BUFS * OUT_COLS], BF16)
            for b in range(OUT_BUFS):
                ob = outbuf[:, b * OUT_COLS:(b + 1) * OUT_COLS]
                bel3 = ob[:, 0:PX_PART * N_SUB].rearrange(
                    "p (g q) -> p g q", q=N_SUB)
                pl3 = ob[:, PX_PART * N_SUB:OUT_COLS].rearrange(
                    "p (g q) -> p g q", q=N_SUB)
                nc.vector.memset(bel3[:, :, 63:64], 1.0)
                nc.vector.memset(pl3[:, :, 0:1], 0.0)
                nc.vector.memset(pl3[:, :, 63:64], 1.0)

            # pl adds + output store are emitted ONE TILE BEHIND the bel
            # copies: Bacc hoists an instruction's extra semaphore waits
            # into event-sems that prefix it in the queue, so if pl0(k)
            # (which needs bel0(k) from ACT) were emitted in the same
            # iteration as the bel h1 CAST(k), the CAST would transitively
            # wait on bel0 too, serializing bel0 -> CAST -> pl1 -> store
            # into one long per-tile chain.  Deferred one tile, every
            # cross-engine wait references work finished an iteration ago.
            tail = None

            def emit_tail(t, ob, bel3, pl3, om0, om1):
                nc.vector.tensor_add(pl3[:, slice(0, GROUPS_PER_MM), 1:63],
                                     bel3[:, slice(0, GROUPS_PER_MM), 1:63],
                                     om0)
                nc.gpsimd.tensor_add(
                    pl3[:, slice(GROUPS_PER_MM, PX_PART), 1:63],
                    bel3[:, slice(GROUPS_PER_MM, PX_PART), 1:63], om1)
                nc.sync.dma_start(out[t], ob)

            for g in range(n_total):
                t = g % n_tiles
                G = g // igrp
                j = (g % n_tiles) % igrp
                # The deferred tail MUST be emitted before ensure_group:
                # the G+2 load recycles the buffer of group G-1, and tile
                # g-1 (last of G-1 at a group head) still has pending pl
                # adds reading that omega buffer.  Emitting the load first
                # would order those reads after the overwrite.
                if tail is not None:
                    emit_tail(*tail)
                    tail = None
                if j == 0:
                    ensure_group(G + 2)
                if g and g % WARM_EVERY == 0:
                    # Refresher: if the HAM gate dropped the PE back to
                    # 1.2 GHz after a pipeline hiccup, ~2.4us of solid
                    # matmul re-arms it; if still warm this costs ~0.9us
                    # of the PE's ~50% idle time.
                    warm_burst(4)

                that = gdat[G][:, j * 128:(j + 1) * 128]

                b = g % OUT_BUFS
                ob = outbuf[:, b * OUT_COLS:(b + 1) * OUT_COLS]
                bel3 = ob[:, 0:PX_PART * N_SUB].rearrange(
                    "p (g q) -> p g q", q=N_SUB)
                pl3 = ob[:, PX_PART * N_SUB:OUT_COLS].rearrange(
                    "p (g q) -> p g q", q=N_SUB)

                slot0 = (2 * g) % 6
                slot1 = (2 * g + 1) % 6
                ps0 = ps_all[:, 512 * slot0:512 * (slot0 + 1)]
                ps1 = ps_all[:, 512 * slot1:512 * (slot1 + 1)]
                nc.tensor.matmul(ps0, that, wmat[:, 0:512])
                nc.tensor.matmul(ps1, that, wmat[:, 512:1024])
                ps30 = ps0.rearrange("p (g q) -> p g q", q=N_SUB)
                ps31 = ps1.rearrange("p (g q) -> p g q", q=N_SUB)

                om = gomg[G]
                om0 = om[:, j * PX_PART:j * PX_PART + GROUPS_PER_MM]
                om0 = bass.AP(om0.tensor, om0.offset, om0.ap + [[0, 62]])
                om1 = om[:, j * PX_PART + GROUPS_PER_MM:(j + 1) * PX_PART]
                om1 = bass.AP(om1.tensor, om1.offset, om1.ap + [[0, 62]])

                # bel cols 0..62 of each group copy PSUM->SBUF (f32->bf16;
                # col 0 comes from the all-zero W column); Pool cannot
                # access PSUM, so ACT takes h0 and DVE takes h1.
                nc.scalar.copy(bel3[:, slice(0, GROUPS_PER_MM), 0:63],
                               ps30[:, :, 0:63])
                nc.vector.tensor_copy(
                    bel3[:, slice(GROUPS_PER_MM, PX_PART), 0:63],
                    ps31[:, :, 0:63])

                tail = (t, ob, bel3, pl3, om0, om1)

            emit_tail(*tail)

    nc.compile()
    return nc


_NC_CACHE: dict[int, bass.Bass] = {}


def _get_program(n_tiles: int) -> bass.Bass:
    if n_tiles not in _NC_CACHE:
        _NC_CACHE[n_tiles] = build_program(n_tiles)
    return _NC_CACHE[n_tiles]


def _prep_core(shard: np.ndarray, n_tiles: int):
    """shard: [n_tiles*2048, 7] bf16 -> (xt [112, n_tiles*128],
    xo [128, n_tiles*16]), both contiguous bf16."""
    v = shard.reshape(n_tiles, 128, PX_PART, N_CH)
    xt = np.ascontiguousarray(
        v[..., :].transpose(2, 3, 0, 1)).reshape(K_ROWS, n_tiles * 128)
    xo = np.ascontiguousarray(
        v[..., 6].transpose(1, 0, 2)).reshape(128, n_tiles * PX_PART)
    return xt, xo


def run_on_cores(x_flat: np.ndarray, **run_kwargs):
    """x_flat: [PX_TOTAL, 7] bf16 (or castable). Returns (bel, pl) each
    [PX_TOTAL, 64] fp32, plus the raw BassKernelResults as third element."""
    if x_flat.dtype != bfloat16:
        x_flat = x_flat.astype(bfloat16)
    nc = _get_program(N_TILES)
    in_maps = []
    for c in range(N_CORES):
        xt, xo = _prep_core(x_flat[c * PX_CORE:(c + 1) * PX_CORE], N_TILES)
        in_maps.append({"xt": xt, "xo": xo})
    rr = run_bass_kernel_spmd(nc, in_maps, core_ids=list(range(N_CORES)),
                              **run_kwargs)
    bel = np.empty((PX_TOTAL, N_SUB), np.float32)
    pl = np.empty((PX_TOTAL, N_SUB), np.float32)
    half = PX_PART * N_SUB
    for c, res in enumerate(rr.results):
        sl = slice(c * PX_CORE, (c + 1) * PX_CORE)
        o = np.asarray(res["out"])          # [N_TILES, 128, 2048] bf16
        bel[sl] = o[:, :, :half].astype(np.float32).reshape(PX_CORE, N_SUB)
        pl[sl] = o[:, :, half:].astype(np.float32).reshape(PX_CORE, N_SUB)
    return bel, pl, rr


def kernel(inputs: np.ndarray):
    inputs = np.ascontiguousarray(np.asarray(inputs, dtype=np.float32))
    b, hh, ww, ch = inputs.shape
    x_flat = inputs.reshape(-1, ch).astype(bfloat16)
    bel, pl, _ = run_on_cores(x_flat)
    return (bel.reshape(b, hh, ww, N_SUB), pl.reshape(b, hh, ww, N_SUB))


# revision 27
# speedup vs baseline: 1.0428x; 1.0045x over previous
"""Trainium2 Bass kernel for BeliefPlausibility (Dempster-Shafer bel/pl maps).

Problem: input [4, 384, 1248, 7] fp32 (6 singleton masses + omega per pixel).
Output: tuple (bel, pl), each [4, 384, 1248, 64] fp32 where, per pixel with
masses m_0..m_5 and omega w:
    bel[q] = sum_c m_c * ((q >> c) & 1)  for q in 1..62;  bel[0]=0, bel[63]=1
    pl[q]  = bel[q] + w                  for q in 1..62;  pl[0]=0,  pl[63]=1

Strategy (pure data parallel over 8 cores, no cross-core communication):
  - The kernel is HBM-bound: the two outputs are 64x the input.  All device
    I/O is bf16 (inputs quantized on host, outputs upcast on host).  All
    values are positive sums (no cancellation), so worst-case relative error
    is ~3 bf16 round-offs (~1e-2), inside the 2e-2 gate.  bf16 (not fp16)
    because fp16's subnormal range [6e-8, 6e-5] would blow up the relative
    error of the smallest singleton masses.
  - Flatten pixels; each core gets 239,616 pixels as 117 supertiles of
    (128 partition-pixels x 16 group-pixels x 7 channels).  The HOST
    pre-transposes each supertile to channels-on-partitions [112, 128]
    (plus a tiny pixel-major omega side tensor), so the device needs NO
    transpose at all: a supertile slice of the group load is directly the
    stationary matmul operand.
  - Per supertile: two bf16 matmuls against a constant [112, 1024]
    membership matrix produce PSUM [128, 512] f32 already in the per-pixel
    bel layout (8 pixel-groups x 64 output columns); ACT copies bel h0 and
    DVE copies bel h1 PSUM->SBUF bf16 (Pool cannot access PSUM); DVE/Pool
    derive pl = bel + omega with zero-stride broadcast APs; one contiguous
    512 KB DMA stores bel|pl together.
  - Engine budget per tile vs the ~1.55us DMA-store cadence: PE 1.47us
    cold / 0.74us warm, ACT ~0.75us, DVE ~1.35us, Pool ~1.37us — the
    output DMA is the pacer.  pl adds + store are emitted one tile behind
    the bel copies so Bacc's hoisted event-sem waits only ever reference
    the previous iteration's work.
  - Input loads ride the ACT HWDGE ring, output stores the SP ring:
    sharing one ring would head-of-line-block loads behind stores still
    waiting on compute semaphores.
  - The PE HAM clock gate runs the PE at 1.2 GHz until it sees ~3.4us of
    sustained activity; a startup burst plus periodic 4-matmul refresher
    bursts (into a dedicated junk PSUM bank) keep it at 2.4 GHz.
"""

import sys

import numpy as np
from ml_dtypes import bfloat16

if "concourse" not in sys.modules:
    try:
        import concourse  # noqa: F401
    except ImportError:
        sys.path.insert(0, "/opt/trn_rl_repo")

import concourse.bacc as bacc
import concourse.bass as bass
import concourse.mybir as mybir
import concourse.tile as tile
from concourse.bass_utils import run_bass_kernel_spmd

F32 = mybir.dt.float32
BF16 = mybir.dt.bfloat16

N_CORES = 8
PX_TOTAL = 4 * 384 * 1248          # 1,916,928 pixels
PX_CORE = PX_TOTAL // N_CORES      # 239,616
PX_PART = 16                       # pixels per partition per supertile
PX_TILE = 128 * PX_PART            # 2048 pixels per supertile
N_TILES = PX_CORE // PX_TILE       # 117
N_CH = 7                           # 6 singletons + omega
N_SUB = 64                         # output positions per pixel
K_ROWS = PX_PART * N_CH            # 112 channel rows
GROUPS_PER_MM = 8                  # pixel-groups covered by one matmul
N_MM = PX_PART // GROUPS_PER_MM    # 2 matmuls per supertile
OUT_COLS = 2 * PX_PART * N_SUB     # 2048: bel block | pl block per tile
IN_GRP = 9                         # supertiles per input DMA (117 = 9*13)
IN_BUFS = 3                        # input group buffers in flight
OUT_BUFS = 22                      # persistent output SBUF buffers
WARM_EVERY = 24                    # tiles between PE warm refresher bursts


def _weight_matrix() -> np.ndarray:
    """[112, 1024]: W[7j+c, 512h+64g+q] = (q>>c)&1 for j=8h+g, q in 1..62,
    c in 0..5.  Columns (g,0) and (g,63) stay zero (written separately)."""
    w = np.zeros((K_ROWS, N_MM * 512), np.float32)
    for h in range(N_MM):
        for g in range(GROUPS_PER_MM):
            j = GROUPS_PER_MM * h + g
            col0 = 512 * h + 64 * g
            for q in range(1, 63):
                for c in range(6):
                    if (q >> c) & 1:
                        w[7 * j + c, col0 + q] = 1.0
    return w


def build_program(n_tiles: int = N_TILES, reps: int = 1) -> bass.Bass:
    igrp = IN_GRP if n_tiles % IN_GRP == 0 else 1
    # Bacc (not plain Bass): its compile() runs generate_event_semaphores,
    # which splits multi-semaphore waits into standalone event-sem
    # instructions (TRN2 allows at most one wait per instruction).
    nc = bacc.Bacc("TRN2")

    # Host-pretransposed inputs: xt[7j+c, 128t+p] = mass c of pixel
    # (t, p, j); xo[p, 16t+j] = omega of pixel (t, p, j).
    xt = nc.dram_tensor("xt", (K_ROWS, n_tiles * 128), BF16,
                        kind="ExternalInput")
    xo = nc.dram_tensor("xo", (128, n_tiles * PX_PART), BF16,
                        kind="ExternalInput")
    out = nc.dram_tensor("out", (n_tiles, 128, OUT_COLS), BF16,
                         kind="ExternalOutput")

    w_dram = nc.inline_tensor(_weight_matrix(), name="wmat")

    n_total = reps * n_tiles
    n_grp = (n_total + igrp - 1) // igrp
    grp_per_pass = n_tiles // igrp

    with tile.TileContext(nc) as tc:
        with (
            tc.tile_pool(name="const", bufs=1) as cpool,
            tc.tile_pool(name="psM", bufs=1, space="PSUM") as psMpool,
            tc.tile_pool(name="psD", bufs=1, space="PSUM") as psDpool,
        ):
            # Persistent input group buffers, cycled manually.  Group K is
            # issued at the head of group K-2 (3 buffers keep K-2..K
            # alive), so data always lands well before the matmuls read it.
            inbuf = cpool.tile([K_ROWS, IN_BUFS * igrp * 128], BF16)
            ombuf = cpool.tile([128, IN_BUFS * igrp * PX_PART], BF16)
            gdat: dict[int, object] = {}
            gomg: dict[int, object] = {}

            def ensure_group(G: int):
                if G >= n_grp or G in gdat:
                    return
                b = G % IN_BUFS
                ib = inbuf[:, b * igrp * 128:(b + 1) * igrp * 128]
                ob = ombuf[:, b * igrp * PX_PART:(b + 1) * igrp * PX_PART]
                Gp = G % grp_per_pass
                nc.scalar.dma_start(
                    ib, xt[:, Gp * igrp * 128:(Gp + 1) * igrp * 128])
                nc.scalar.dma_start(
                    ob, xo[:, Gp * igrp * PX_PART:(Gp + 1) * igrp * PX_PART])
                gdat[G] = ib
                gomg[G] = ob

            ensure_group(0)
            ensure_group(1)

            # Stage the weights through an ACT copy (f32 -> bf16; all
            # values are exact 0/1).
            wstage = cpool.tile([K_ROWS, N_MM * 512], F32)
            nc.sync.dma_start(wstage[:], w_dram[:])
            wmat = cpool.tile([K_ROWS, N_MM * 512], BF16)
            nc.scalar.copy(wmat[:], wstage[:])

            # Six matmul PSUM slots cycled manually (slot reuse distance =
            # 3 tiles, so the slot-release WAR always references copies
            # finished long ago).  Even slots are read by ACT, odd slots
            # by DVE — one reader engine per slot keeps matmul sync-waits
            # merged.  `dum` is a dedicated junk bank for HAM warm bursts
            # (same-engine WAW only, so the bursts never wait on anything).
            ps_all = psMpool.tile([128, 6 * 512], F32)
            dum = psDpool.tile([128, 512], F32)

            def warm_burst(n: int):
                for _ in range(n):
                    nc.tensor.matmul(dum[:], wmat[:, 0:128], wmat[:, 0:512])

            warm_burst(12)

            # Persistent output buffers (manually cycled slices).  Constant
            # columns are initialized once, outside the loop; the per-tile
            # copies/adds never touch them.
            outbuf = cpool.tile([128, OUT_BUFS * OUT_COLS], BF16)
            for b in range(OUT_BUFS):
                ob = outbuf[:, b * OUT_COLS:(b + 1) * OUT_COLS]
                bel3 = ob[:, 0:PX_PART * N_SUB].rearrange(
                    "p (g q) -> p g q", q=N_SUB)
                pl3 = ob[:, PX_PART * N_SUB:OUT_COLS].rearrange(
                    "p (g q) -> p g q", q=N_SUB)
                nc.vector.memset(bel3[:, :, 63:64], 1.0)
                nc.vector.memset(pl3[:, :, 0:1], 0.0)
                nc.vector.memset(pl3[:, :, 63:64], 1.0)

            # pl adds + output store are emitted ONE TILE BEHIND the bel
            # copies: Bacc hoists an instruction's extra semaphore waits
            # into event-sems that prefix it in the queue, so if pl0(k)
            # (which needs bel0(k) from ACT) were emitted in the same
            # iteration as the bel h1 CAST(k), the CAST would transitively
            # wait on bel0 too, serializing bel0 -> CAST -> pl1 -> store
            # into one long per-tile chain.  Deferred one tile, every
            # cross-engine wait references work finished an iteration ago.
            tail = None

            def emit_tail(t, ob, bel3, pl3, om0, om1):
                nc.vector.tensor_add(pl3[:, slice(0, GROUPS_PER_MM), 1:63],
                                     bel3[:, slice(0, GROUPS_PER_MM), 1:63],
                                     om0)
                nc.gpsimd.tensor_add(
                    pl3[:, slice(GROUPS_PER_MM, PX_PART), 1:63],
                    bel3[:, slice(GROUPS_PER_MM, PX_PART), 1:63], om1)
                nc.sync.dma_start(out[t], ob)

            for g in range(n_total):
                t = g % n_tiles
                G = g // igrp
                j = (g % n_tiles) % igrp
                # The deferred tail MUST be emitted before ensure_group:
                # the G+2 load recycles the buffer of group G-1, and tile
                # g-1 (last of G-1 at a group head) still has pending pl
                # adds reading that omega buffer.  Emitting the load first
                # would order those reads after the overwrite.
                if tail is not None:
                    emit_tail(*tail)
                    tail = None
                if j == 0:
                    ensure_group(G + 2)
                if g and g % WARM_EVERY == 0:
                    # Refresher: if the HAM gate dropped the PE back to
                    # 1.2 GHz after a pipeline hiccup, ~2.4us of solid
                    # matmul re-arms it; if still warm this costs ~0.9us
                    # of the PE's ~50% idle time.
                    warm_burst(6)

                that = gdat[G][:, j * 128:(j + 1) * 128]

                b = g % OUT_BUFS
                ob = outbuf[:, b * OUT_COLS:(b + 1) * OUT_COLS]
                bel3 = ob[:, 0:PX_PART * N_SUB].rearrange(
                    "p (g q) -> p g q", q=N_SUB)
                pl3 = ob[:, PX_PART * N_SUB:OUT_COLS].rearrange(
                    "p (g q) -> p g q", q=N_SUB)

                slot0 = (2 * g) % 6
                slot1 = (2 * g + 1) % 6
                ps0 = ps_all[:, 512 * slot0:512 * (slot0 + 1)]
                ps1 = ps_all[:, 512 * slot1:512 * (slot1 + 1)]
                nc.tensor.matmul(ps0, that, wmat[:, 0:512])
                nc.tensor.matmul(ps1, that, wmat[:, 512:1024])
                ps30 = ps0.rearrange("p (g q) -> p g q", q=N_SUB)
                ps31 = ps1.rearrange("p (g q) -> p g q", q=N_SUB)

                om = gomg[G]
                om0 = om[:, j * PX_PART:j * PX_PART + GROUPS_PER_MM]
                om0 = bass.AP(om0.tensor, om0.offset, om0.ap + [[0, 62]])
                om1 = om[:, j * PX_PART + GROUPS_PER_MM:(j + 1) * PX_PART]
                om1 = bass.AP(om1.tensor, om1.offset, om1.ap + [[0, 62]])

                # bel cols 0..62 of each group copy PSUM->SBUF (f32->bf16;
                # col 0 comes from the all-zero W column); Pool cannot
                # access PSUM, so ACT takes h0 and DVE takes h1.
                nc.scalar.copy(bel3[:, slice(0, GROUPS_PER_MM), 0:63],
                               ps30[:, :, 0:63])
                nc.vector.tensor_copy(
                    bel3[:, slice(GROUPS_PER_MM, PX_PART), 0:63],
                    ps31[:, :, 0:63])

                tail = (t, ob, bel3, pl3, om0, om1)

            emit_tail(*tail)

    nc.compile()
    return nc


_NC_CACHE: dict[int, bass.Bass] = {}


def _get_program(n_tiles: int) -> bass.Bass:
    if n_tiles not in _NC_CACHE:
        _NC_CACHE[n_tiles] = build_program(n_tiles)
    return _NC_CACHE[n_tiles]


def _prep_core(shard: np.ndarray, n_tiles: int):
    """shard: [n_tiles*2048, 7] bf16 -> (xt [112, n_tiles*128],
    xo [128, n_tiles*16]), both contiguous bf16."""
    v = shard.reshape(n_tiles, 128, PX_PART, N_CH)
    xt = np.ascontiguousarray(
        v[..., :].transpose(2, 3, 0, 1)).reshape(K_ROWS, n_tiles * 128)
    xo = np.ascontiguousarray(
        v[..., 6].transpose(1, 0, 2)).reshape(128, n_tiles * PX_PART)
    return xt, xo


def run_on_cores(x_flat: np.ndarray, **run_kwargs):
    """x_flat: [PX_TOTAL, 7] bf16 (or castable). Returns (bel, pl) each
    [PX_TOTAL, 64] fp32, plus the raw BassKernelResults as third element."""
    if x_flat.dtype != bfloat16:
        x_flat = x_flat.astype(bfloat16)
    nc = _get_program(N_TILES)
    in_maps = []
    for c in range(N_CORES):
        xt, xo = _prep_core(x_flat[c * PX_CORE:(c + 1) * PX_CORE], N_TILES)
        in_maps.append({"xt": xt, "xo": xo})
    rr = run_bass_kernel_spmd(nc, in_maps, core_ids=list(range(N_CORES)),
                              **run_kwargs)
    bel = np.empty((PX_TOTAL, N_SUB), np.float32)
    pl = np.empty((PX_TOTAL, N_SUB), np.float32)
    half = PX_PART * N_SUB
    for c, res in enumerate(rr.results):
        sl = slice(c * PX_CORE, (c + 1) * PX_CORE)
        o = np.asarray(res["out"])          # [N_TILES, 128, 2048] bf16
        bel[sl] = o[:, :, :half].astype(np.float32).reshape(PX_CORE, N_SUB)
        pl[sl] = o[:, :, half:].astype(np.float32).reshape(PX_CORE, N_SUB)
    return bel, pl, rr


def kernel(inputs: np.ndarray):
    inputs = np.ascontiguousarray(np.asarray(inputs, dtype=np.float32))
    b, hh, ww, ch = inputs.shape
    x_flat = inputs.reshape(-1, ch).astype(bfloat16)
    bel, pl, _ = run_on_cores(x_flat)
    return (bel.reshape(b, hh, ww, N_SUB), pl.reshape(b, hh, ww, N_SUB))


# revision 33
# speedup vs baseline: 1.2597x; 1.2079x over previous
"""Trainium2 Bass kernel for BeliefPlausibility (Dempster-Shafer bel/pl maps).

Problem: input [4, 384, 1248, 7] fp32 (6 singleton masses + omega per pixel).
Output: tuple (bel, pl), each [4, 384, 1248, 64] fp32 where, per pixel with
masses m_0..m_5 and omega w:
    bel[q] = sum_c m_c * ((q >> c) & 1)  for q in 1..62;  bel[0]=0, bel[63]=1
    pl[q]  = bel[q] + w                  for q in 1..62;  pl[0]=0,  pl[63]=1

Strategy (pure data parallel over 8 cores, no cross-core communication):
  - The kernel is HBM-bound: the two outputs are 64x the input.  All device
    I/O is bf16 (inputs quantized on host, outputs upcast on host).  All
    values are positive sums (no cancellation), so worst-case relative error
    is ~3 bf16 round-offs (~1e-2), inside the 2e-2 gate.  bf16 (not fp16)
    because fp16's subnormal range [6e-8, 6e-5] would blow up the relative
    error of the smallest singleton masses.
  - Flatten pixels; each core gets 239,616 pixels as 117 supertiles of
    (128 partition-pixels x 16 group-pixels x 7 channels).  The HOST
    pre-transposes each supertile to channels-on-partitions [112, 128]
    (plus a tiny pixel-major omega side tensor), so the device needs NO
    transpose at all: a supertile slice of the group load is directly the
    stationary matmul operand.
  - Per supertile: two bf16 matmuls against a constant [112, 1024]
    membership matrix produce PSUM [128, 512] f32 already in the per-pixel
    bel layout (8 pixel-groups x 64 output columns); ACT copies bel h0 and
    DVE copies bel h1 PSUM->SBUF bf16 (Pool cannot access PSUM); DVE/Pool
    derive pl = bel + omega with zero-stride broadcast APs; one contiguous
    512 KB DMA stores bel|pl together.
  - Engine budget per tile vs the ~1.55us DMA-store cadence: PE 1.47us
    cold / 0.74us warm, ACT ~0.75us, DVE ~1.35us, Pool ~1.37us — the
    output DMA is the pacer.  pl adds + store are emitted one tile behind
    the bel copies so Bacc's hoisted event-sem waits only ever reference
    the previous iteration's work.
  - Input loads ride the ACT HWDGE ring, output stores the SP ring:
    sharing one ring would head-of-line-block loads behind stores still
    waiting on compute semaphores.
  - The PE HAM clock gate runs the PE at 1.2 GHz until it sees ~3.4us of
    sustained activity; a startup burst plus periodic 4-matmul refresher
    bursts (into a dedicated junk PSUM bank) keep it at 2.4 GHz.
"""

import sys

import numpy as np
from ml_dtypes import bfloat16

if "concourse" not in sys.modules:
    try:
        import concourse  # noqa: F401
    except ImportError:
        sys.path.insert(0, "/opt/trn_rl_repo")

import concourse.bacc as bacc
import concourse.bass as bass
import concourse.mybir as mybir
import concourse.tile as tile
from concourse.bass_utils import run_bass_kernel_spmd

F32 = mybir.dt.float32
BF16 = mybir.dt.bfloat16

N_CORES = 8
PX_TOTAL = 4 * 384 * 1248          # 1,916,928 pixels
PX_CORE = PX_TOTAL // N_CORES      # 239,616
PX_PART = 16                       # pixels per partition per supertile
PX_TILE = 128 * PX_PART            # 2048 pixels per supertile
N_TILES = PX_CORE // PX_TILE       # 117
N_CH = 7                           # 6 singletons + omega
N_SUB = 64                         # output positions per pixel
K_ROWS = PX_PART * N_CH            # 112 channel rows
GROUPS_PER_MM = 8                  # pixel-groups covered by one matmul
N_MM = PX_PART // GROUPS_PER_MM    # 2 matmuls per supertile
OUT_COLS = 2 * PX_PART * N_SUB     # 2048: bel block | pl block per tile
IN_GRP = 9                         # supertiles per input DMA (117 = 9*13)
IN_BUFS = 3                        # input group buffers in flight
OUT_BUFS = 22                      # persistent output SBUF buffers
WARM_EVERY = 24                    # tiles between PE warm refresher bursts


def _weight_matrix() -> np.ndarray:
    """[112, 1024]: W[7j+c, 512h+64g+q] = (q>>c)&1 for j=8h+g, q in 1..62,
    c in 0..5.  Columns (g,0) and (g,63) stay zero (written separately)."""
    w = np.zeros((K_ROWS, N_MM * 512), np.float32)
    for h in range(N_MM):
        for g in range(GROUPS_PER_MM):
            j = GROUPS_PER_MM * h + g
            col0 = 512 * h + 64 * g
            for q in range(1, 63):
                for c in range(6):
                    if (q >> c) & 1:
                        w[7 * j + c, col0 + q] = 1.0
    return w


def build_program(n_tiles: int = N_TILES, reps: int = 1,
                  do_copies: bool = True, do_pl: bool = True,
                  do_out: bool = True) -> bass.Bass:
    igrp = IN_GRP if n_tiles % IN_GRP == 0 else 1
    # Bacc (not plain Bass): its compile() runs generate_event_semaphores,
    # which splits multi-semaphore waits into standalone event-sem
    # instructions (TRN2 allows at most one wait per instruction).
    nc = bacc.Bacc("TRN2")

    # Host-pretransposed inputs: xt[7j+c, 128t+p] = mass c of pixel
    # (t, p, j); xo[p, 16t+j] = omega of pixel (t, p, j).
    xt = nc.dram_tensor("xt", (K_ROWS, n_tiles * 128), BF16,
                        kind="ExternalInput")
    xo = nc.dram_tensor("xo", (128, n_tiles * PX_PART), BF16,
                        kind="ExternalInput")
    out = nc.dram_tensor("out", (128, n_tiles * OUT_COLS), BF16,
                         kind="ExternalOutput")

    w_dram = nc.inline_tensor(_weight_matrix(), name="wmat")

    n_total = reps * n_tiles
    n_grp = (n_total + igrp - 1) // igrp
    grp_per_pass = n_tiles // igrp

    with tile.TileContext(nc) as tc:
        with (
            tc.tile_pool(name="const", bufs=1) as cpool,
            tc.tile_pool(name="psM", bufs=1, space="PSUM") as psMpool,
            tc.tile_pool(name="psD", bufs=1, space="PSUM") as psDpool,
        ):
            # Persistent input group buffers, cycled manually.  Group K is
            # issued at the head of group K-2 (3 buffers keep K-2..K
            # alive), so data always lands well before the matmuls read it.
            inbuf = cpool.tile([K_ROWS, IN_BUFS * igrp * 128], BF16)
            ombuf = cpool.tile([128, IN_BUFS * igrp * PX_PART], BF16)
            gdat: dict[int, object] = {}
            gomg: dict[int, object] = {}

            def ensure_group(G: int):
                if G >= n_grp or G in gdat:
                    return
                b = G % IN_BUFS
                ib = inbuf[:, b * igrp * 128:(b + 1) * igrp * 128]
                ob = ombuf[:, b * igrp * PX_PART:(b + 1) * igrp * PX_PART]
                Gp = G % grp_per_pass
                nc.scalar.dma_start(
                    ib, xt[:, Gp * igrp * 128:(Gp + 1) * igrp * 128])
                nc.scalar.dma_start(
                    ob, xo[:, Gp * igrp * PX_PART:(Gp + 1) * igrp * PX_PART])
                gdat[G] = ib
                gomg[G] = ob

            ensure_group(0)
            ensure_group(1)

            # Stage the weights through an ACT copy (f32 -> bf16; all
            # values are exact 0/1).
            wstage = cpool.tile([K_ROWS, N_MM * 512], F32)
            nc.sync.dma_start(wstage[:], w_dram[:])
            wmat = cpool.tile([K_ROWS, N_MM * 512], BF16)
            nc.scalar.copy(wmat[:], wstage[:])

            # Six matmul PSUM slots cycled manually (slot reuse distance =
            # 3 tiles, so the slot-release WAR always references copies
            # finished long ago).  Even slots are read by ACT, odd slots
            # by DVE — one reader engine per slot keeps matmul sync-waits
            # merged.  `dum` is a dedicated junk bank for HAM warm bursts
            # (same-engine WAW only, so the bursts never wait on anything).
            ps_all = psMpool.tile([128, 6 * 512], F32)
            dum = psDpool.tile([128, 512], F32)

            def warm_burst(n: int):
                for _ in range(n):
                    nc.tensor.matmul(dum[:], wmat[:, 0:128], wmat[:, 0:512])

            warm_burst(12)

            # Persistent output buffers (manually cycled slices).  Constant
            # columns are initialized once, outside the loop; the per-tile
            # copies/adds never touch them.
            outbuf = cpool.tile([128, OUT_BUFS * OUT_COLS], BF16)
            for b in range(OUT_BUFS):
                ob = outbuf[:, b * OUT_COLS:(b + 1) * OUT_COLS]
                bel3 = ob[:, 0:PX_PART * N_SUB].rearrange(
                    "p (g q) -> p g q", q=N_SUB)
                pl3 = ob[:, PX_PART * N_SUB:OUT_COLS].rearrange(
                    "p (g q) -> p g q", q=N_SUB)
                nc.vector.memset(bel3[:, :, 63:64], 1.0)
                nc.vector.memset(pl3[:, :, 0:1], 0.0)
                nc.vector.memset(pl3[:, :, 63:64], 1.0)

            # 3-tile blocks: the six PSUM banks hold exactly three
            # supertiles of matmul output (h0 in even banks 0/2/4, h1 in
            # odd banks 1/3/5), and each downstream engine touches a block
            # with ONE strided 4D instruction.  The runtime's event-sem
            # lowering serializes each cross-engine hop conservatively
            # (measured: per-tile emission paces at ~2.7us/tile even with
            # no DMA at all), so amortizing the hop chain over 3 tiles is
            # worth ~1.5x.  pl adds + the block store are emitted one
            # BLOCK behind the bel copies for the same reason (hoisted
            # waits must only reference the previous block's work).
            BLK = 3
            assert n_total % BLK == 0 and igrp % BLK == 0
            tail = None

            def blk_views(lo):
                """4D views [p, tile, group, col] over a 3-tile block of
                outbuf: (bel_h0, bel_h1, pl_h0, pl_h1)."""
                ob = outbuf[:, lo:lo + BLK * OUT_COLS].rearrange(
                    "p (t c) -> p t c", c=OUT_COLS)
                bel = ob[:, :, 0:PX_PART * N_SUB].rearrange(
                    "p t (g q) -> p t g q", q=N_SUB)
                pl = ob[:, :, PX_PART * N_SUB:OUT_COLS].rearrange(
                    "p t (g q) -> p t g q", q=N_SUB)
                return (bel[:, :, 0:GROUPS_PER_MM, :],
                        bel[:, :, GROUPS_PER_MM:PX_PART, :],
                        pl[:, :, 0:GROUPS_PER_MM, :],
                        pl[:, :, GROUPS_PER_MM:PX_PART, :])

            def emit_tail(t0, lo, om0, om1):
                b0, b1, p0, p1 = blk_views(lo)
                if do_pl:
                    nc.vector.tensor_add(p0[:, :, :, 1:63],
                                         b0[:, :, :, 1:63], om0)
                    nc.gpsimd.tensor_add(p1[:, :, :, 1:63],
                                         b1[:, :, :, 1:63], om1)
                if do_out:
                    nc.sync.dma_start(
                        out[:, t0 * OUT_COLS:(t0 + BLK) * OUT_COLS],
                        outbuf[:, lo:lo + BLK * OUT_COLS])

            ps6 = ps_all[:].rearrange("p (t x) -> p t x", x=2 * 512)

            for blk in range(n_total // BLK):
                g0 = blk * BLK
                t0 = g0 % n_tiles
                G = g0 // igrp
                j0 = (g0 % n_tiles) % igrp
                if tail is not None:
                    emit_tail(*tail)
                    tail = None
                if j0 == 0:
                    ensure_group(G + 2)
                if blk and g0 % WARM_EVERY < BLK:
                    warm_burst(6)

                lo = (blk % (OUT_BUFS // BLK)) * BLK * OUT_COLS

                for i in range(BLK):
                    that = gdat[G][:, (j0 + i) * 128:(j0 + i + 1) * 128]
                    ps0 = ps_all[:, 1024 * i:1024 * i + 512]
                    ps1 = ps_all[:, 1024 * i + 512:1024 * (i + 1)]
                    nc.tensor.matmul(ps0, that, wmat[:, 0:512])
                    nc.tensor.matmul(ps1, that, wmat[:, 512:1024])

                # [p, tile, group, col] views over PSUM banks 0/2/4 (h0)
                # and 1/3/5 (h1).
                ph0 = ps6[:, :, 0:512].rearrange(
                    "p t (g q) -> p t g q", q=N_SUB)
                ph1 = ps6[:, :, 512:1024].rearrange(
                    "p t (g q) -> p t g q", q=N_SUB)

                om = gomg[G]
                omv = om[:, j0 * PX_PART:(j0 + BLK) * PX_PART].rearrange(
                    "p (t g) -> p t g", g=PX_PART)
                om0 = omv[:, :, 0:GROUPS_PER_MM]
                om0 = bass.AP(om0.tensor, om0.offset, om0.ap + [[0, 62]])
                om1 = omv[:, :, GROUPS_PER_MM:PX_PART]
                om1 = bass.AP(om1.tensor, om1.offset, om1.ap + [[0, 62]])

                b0, b1, p0, p1 = blk_views(lo)
                if do_copies:
                    nc.scalar.copy(b0[:, :, :, 0:63], ph0[:, :, :, 0:63])
                    nc.vector.tensor_copy(b1[:, :, :, 0:63],
                                          ph1[:, :, :, 0:63])

                tail = (t0, lo, om0, om1)

            emit_tail(*tail)

    nc.compile()
    return nc


_NC_CACHE: dict[int, bass.Bass] = {}


def _get_program(n_tiles: int) -> bass.Bass:
    if n_tiles not in _NC_CACHE:
        _NC_CACHE[n_tiles] = build_program(n_tiles)
    return _NC_CACHE[n_tiles]


def _prep_core(shard: np.ndarray, n_tiles: int):
    """shard: [n_tiles*2048, 7] bf16 -> (xt [112, n_tiles*128],
    xo [128, n_tiles*16]), both contiguous bf16."""
    v = shard.reshape(n_tiles, 128, PX_PART, N_CH)
    xt = np.ascontiguousarray(
        v[..., :].transpose(2, 3, 0, 1)).reshape(K_ROWS, n_tiles * 128)
    xo = np.ascontiguousarray(
        v[..., 6].transpose(1, 0, 2)).reshape(128, n_tiles * PX_PART)
    return xt, xo


def run_on_cores(x_flat: np.ndarray, **run_kwargs):
    """x_flat: [PX_TOTAL, 7] bf16 (or castable). Returns (bel, pl) each
    [PX_TOTAL, 64] fp32, plus the raw BassKernelResults as third element."""
    if x_flat.dtype != bfloat16:
        x_flat = x_flat.astype(bfloat16)
    nc = _get_program(N_TILES)
    in_maps = []
    for c in range(N_CORES):
        xt, xo = _prep_core(x_flat[c * PX_CORE:(c + 1) * PX_CORE], N_TILES)
        in_maps.append({"xt": xt, "xo": xo})
    rr = run_bass_kernel_spmd(nc, in_maps, core_ids=list(range(N_CORES)),
                              **run_kwargs)
    bel = np.empty((PX_TOTAL, N_SUB), np.float32)
    pl = np.empty((PX_TOTAL, N_SUB), np.float32)
    half = PX_PART * N_SUB
    for c, res in enumerate(rr.results):
        sl = slice(c * PX_CORE, (c + 1) * PX_CORE)
        o = np.asarray(res["out"]).reshape(128, N_TILES, OUT_COLS)
        o = o.transpose(1, 0, 2)            # [N_TILES, 128, 2048] bf16
        bel[sl] = o[:, :, :half].astype(np.float32).reshape(PX_CORE, N_SUB)
        pl[sl] = o[:, :, half:].astype(np.float32).reshape(PX_CORE, N_SUB)
    return bel, pl, rr


def kernel(inputs: np.ndarray):
    inputs = np.ascontiguousarray(np.asarray(inputs, dtype=np.float32))
    b, hh, ww, ch = inputs.shape
    x_flat = inputs.reshape(-1, ch).astype(bfloat16)
    bel, pl, _ = run_on_cores(x_flat)
    return (bel.reshape(b, hh, ww, N_SUB), pl.reshape(b, hh, ww, N_SUB))
